# revision 1
# baseline (speedup 1.0000x reference)
"""BoT multi-head attention block (QKV proj + content/position attention +
out-proj + residual + LayerNorm) on 8 Trainium2 NeuronCores.

Sharding: tensor-parallel over heads (4 heads/core) x batch (2 batches, 4
cores each).  Each core computes q/k/v projections for its 256 feature
columns, full attention for its 4 heads, and a partial out-projection;
partials are summed with row-chunked ReduceScatters over each 4-core batch
group (overlapped with attention of later chunks), after which each core
applies residual + LayerNorm to its 4x128 rows.

Layout trick: attention logits are computed TRANSPOSED (j on partitions, i
free) so the softmax numerator matmul (P^T moving, V stationary) needs no
transpose of the probability matrix; an extra all-ones column in the
stationary V supplies the softmax denominator for free.  Host passes x and
pos pre-transposed.  Projections/out-proj run in fp32r, attention matmuls
in bf16 (1 cycle/col vs 2 for fp32r).
"""

import contextlib
import os
import sys

os.environ.setdefault("MYCRO_LOCAL_CACHE", "1")
for _p in ("/opt/trn_rl_repo",):
    if os.path.isdir(_p) and _p not in sys.path:
        sys.path.append(_p)

import ml_dtypes
import numpy as np

import concourse.bass as bass
from concourse import bacc
import concourse.mybir as mybir
import concourse.tile as tile
from concourse.bass_utils import run_bass_kernel_spmd

FP = mybir.dt.float32
FPR = mybir.dt.float32r
BF = mybir.dt.bfloat16
AF = mybir.ActivationFunctionType

B, N, D, H = 2, 2048, 1024, 16
NCORES = 8
GRP = 4                # cores per batch group
HPC = H // GRP         # heads per core = 4
C = D // GRP           # feature cols per core = 256
R = N // GRP           # output rows per core = 512
DH = D // H            # head dim = 64
SCALE = DH ** -0.5
LN_EPS = 1e-5

NT = N // 128          # 16 row tiles
KD = D // 128          # 8 contraction tiles over D
NS = N // 512          # 4 i-slices

ATT_DT = BF            # dtype of attention matmul operands
PROJ_DT = BF           # dtype of projection inputs (xT, wq/wk/wv)


def build():
    nc = bacc.Bacc("TRN2", target_bir_lowering=False, num_devices=NCORES)

    # fp32r-typed inputs are plain fp32 bits; typing them fp32r lets HWDGE
    # load them with no cast while satisfying the fp32r-producer rule.
    xT_t = nc.dram_tensor("xT", [D, N], PROJ_DT, kind="ExternalInput")
    posT_t = nc.dram_tensor("posT", [C, N], FP, kind="ExternalInput")
    wq_t = nc.dram_tensor("wq", [D, C], PROJ_DT, kind="ExternalInput")
    wk_t = nc.dram_tensor("wk", [D, C], PROJ_DT, kind="ExternalInput")
    wv_t = nc.dram_tensor("wv", [D, C], PROJ_DT, kind="ExternalInput")
    wo_t = nc.dram_tensor("wo", [C, D], FPR, kind="ExternalInput")
    res_t = nc.dram_tensor("resid", [R, D], FP, kind="ExternalInput")
    g_t = nc.dram_tensor("ln_g", [D], FP, kind="ExternalInput")
    bt_t = nc.dram_tensor("ln_b", [D], FP, kind="ExternalInput")
    out_t = nc.dram_tensor("out", [R, D], FP, kind="ExternalOutput")

    res_tiles = res_t.ap().rearrange("(t p) d -> t p d", p=128)
    out_tiles = out_t.ap().rearrange("(t p) d -> t p d", p=128)

    def bcast_ap(ap, parts):
        return bass.AP(tensor=ap.tensor, offset=ap.offset,
                       ap=[[0, parts]] + list(ap.ap))

    with tile.TileContext(nc) as tc, contextlib.ExitStack() as ctx:
        persist = ctx.enter_context(tc.tile_pool(name="persist", bufs=1))
        attnp = ctx.enter_context(tc.tile_pool(name="attnp", bufs=1))
        psP = ctx.enter_context(tc.tile_pool(name="psP", bufs=1, space="PSUM"))
        psO = ctx.enter_context(tc.tile_pool(name="psO", bufs=3, space="PSUM"))
        psC = ctx.enter_context(tc.tile_pool(name="psC", bufs=2, space="PSUM"))
        dram = ctx.enter_context(tc.tile_pool(name="dram", bufs=1, space="DRAM"))

        ones64 = persist.tile([1, DH], FP, tag="ones64")
        nc.vector.memset(ones64, 1.0)
        onescol = persist.tile([128, 1], FP, tag="onescol")
        nc.vector.memset(onescol, 1.0)

        sbA = ctx.enter_context(tc.tile_pool(name="sbA", bufs=3))

        # ---------------- phase 1-2: load (pre-transposed on host), project
        ph12_ctx = contextlib.ExitStack()
        p12 = ph12_ctx.enter_context(tc.tile_pool(name="ph12", bufs=1))

        wq_sb = p12.tile([128, KD, C], PROJ_DT, tag="wq")
        wk_sb = p12.tile([128, KD, C], PROJ_DT, tag="wk")
        wv_sb = p12.tile([128, KD, C], PROJ_DT, tag="wv")
        xT_sb = p12.tile([128, KD, N], PROJ_DT, tag="xT")
        xT_src = xT_t.ap().rearrange("(k p) n -> p k n", p=128)
        nc.sync.dma_start(out=wq_sb, in_=wq_t.ap().rearrange("(k p) c -> p k c", p=128))
        for k in range(KD):
            nc.sync.dma_start(out=xT_sb[:, k, :], in_=xT_src[:, k, :])
        nc.sync.dma_start(out=wk_sb, in_=wk_t.ap().rearrange("(k p) c -> p k c", p=128))
        nc.sync.dma_start(out=wv_sb, in_=wv_t.ap().rearrange("(k p) c -> p k c", p=128))
        xT = [xT_sb[:, k, :] for k in range(KD)]

        posT_sb = p12.tile([128, 2, N], FP, tag="posT")
        nc.sync.dma_start(out=posT_sb,
                          in_=posT_t.ap().rearrange("(m p) n -> p m n", p=128))
        posT = [posT_sb[:, m, :] for m in range(2)]

        # projections: qT/kpT [128 c, N] (head pair hp at rows 64*(h%2))
        qT = [attnp.tile([128, N], ATT_DT, name=f"qT{m}", tag=f"qT{m}") for m in range(2)]
        kpT = [attnp.tile([128, N], ATT_DT, name=f"kpT{m}", tag=f"kpT{m}") for m in range(2)]
        V = [attnp.tile([128, HPC, DH + 1], ATT_DT, name=f"V{t}", tag=f"V{t}")
             for t in range(NT)]

        def proj_qkp(m):
            for s in range(NS):
                q_ps = psP.tile([128, 512], FP, tag="ps", name="q_ps")
                for k in range(KD):
                    nc.tensor.matmul(q_ps, wq_sb[:, k, m * 128:(m + 1) * 128],
                                     xT[k][:, s * 512:(s + 1) * 512],
                                     start=(k == 0), stop=(k == KD - 1))
                nc.vector.tensor_copy(out=qT[m][:, s * 512:(s + 1) * 512], in_=q_ps)
            for s in range(NS):
                kp_ps = psP.tile([128, 512], FP, tag="ps", name="kp_ps")
                for k in range(KD):
                    nc.tensor.matmul(kp_ps, wk_sb[:, k, m * 128:(m + 1) * 128],
                                     xT[k][:, s * 512:(s + 1) * 512],
                                     start=(k == 0), stop=(k == KD - 1))
                nc.vector.tensor_add(out=kpT[m][:, s * 512:(s + 1) * 512],
                                     in0=kp_ps, in1=posT[m][:, s * 512:(s + 1) * 512])

        proj_qkp(0)
        for t in range(NT):
            v_ps = psP.tile([128, C], FP, tag="ps", name="v_ps")
            for k in range(KD):
                nc.tensor.matmul(v_ps, xT[k][:, t * 128:(t + 1) * 128], wv_sb[:, k, :],
                                 start=(k == 0), stop=(k == KD - 1))
            nc.vector.tensor_copy(out=V[t][:, :, 0:DH],
                                  in_=v_ps.rearrange("p (h d) -> p h d", h=HPC))
            nc.vector.tensor_copy(out=V[t][:, :, DH:DH + 1],
                                  in_=onescol.broadcast_to([128, HPC, 1]))

        # ---------------- phases 3-4 interleaved per i-slice s -------------
        pools = {}

        wo_sb = persist.tile([128, 2, D], FPR, tag="wo")
        nc.sync.dma_start(out=wo_sb, in_=wo_t.ap().rearrange("(k p) d -> p k d", p=128))
        g_sb = persist.tile([128, D], FP, tag="g")
        b_sb = persist.tile([128, D], FP, tag="b")
        nc.gpsimd.dma_start(out=g_sb, in_=bcast_ap(g_t.ap(), 128))
        nc.gpsimd.dma_start(out=b_sb, in_=bcast_ap(bt_t.ap(), 128))
        eps_sb = persist.tile([128, 1], FP, tag="eps")
        nc.vector.memset(eps_sb, LN_EPS)

        OT = [attnp.tile([128, N], FPR, name=f"OT{m}", tag=f"OT{m}") for m in range(2)]
        OTU = [attnp.tile([128, N], FP, name=f"OTU{m}", tag=f"OTU{m}") for m in range(2)]
        oph = [dram.tile([R, D], FP, name=f"oph{s}", tag=f"oph{s}") for s in range(NS)]
        rsh = [dram.tile([128, D], FP, name=f"rsh{s}", tag=f"rsh{s}") for s in range(NS)]

        def attention(s, hp):
            ot_e = psO.tile([128, 512], FP, tag="ot", name="ot_e")
            ot_o = psO.tile([128, 512], FP, tag="ot", name="ot_o")
            for jt in range(NT):
                st = psC.tile([128, 1024], FP, tag="st", name="st")
                nc.tensor.matmul(st[:, 0:512],
                                 kpT[hp][0:64, jt * 128:(jt + 1) * 128],
                                 qT[hp][0:64, s * 512:(s + 1) * 512],
                                 start=True, stop=True)
                nc.tensor.matmul(st[:, 512:1024],
                                 kpT[hp][64:128, jt * 128:(jt + 1) * 128],
                                 qT[hp][64:128, s * 512:(s + 1) * 512],
                                 start=True, stop=True)
                ste = sbA.tile([128, 1024], ATT_DT, tag="ste", name="ste")
                nc.scalar.activation(out=ste, in_=st, func=AF.Exp, scale=SCALE)
                nc.tensor.matmul(ot_e[0:DH + 1, :], V[jt][:, 2 * hp, :],
                                 ste[:, 0:512],
                                 start=(jt == 0), stop=(jt == NT - 1))
                nc.tensor.matmul(ot_o[0:DH + 1, :], V[jt][:, 2 * hp + 1, :],
                                 ste[:, 512:1024],
                                 start=(jt == 0), stop=(jt == NT - 1))
            # evacuate PSUM immediately: unnormalized OT rows + colsum row to
            # SBUF (releases the ot accumulators within ~1.5us); the actual
            # softmax division happens later, overlapped with the next slice
            jobs = []
            for par, ot in ((0, ot_e), (1, ot_o)):
                csrow = sbA.tile([1, 512], FP, tag="csrow", name="csrow", bufs=8)
                nc.vector.tensor_copy(out=csrow, in_=ot[DH:DH + 1, :])
                dst = OT[hp][par * 64:par * 64 + DH, s * 512:(s + 1) * 512]
                dstu = OTU[hp][par * 64:par * 64 + DH, s * 512:(s + 1) * 512]
                nc.vector.tensor_copy(out=dstu, in_=ot[0:DH, :])
                jobs.append((dst, dstu, csrow, par))
            return jobs

        def normalize(jobs):
            for dst, dstu, csrow, par in jobs:
                csr = sbA.tile([1, 512], FP, tag="csr", name="csr", bufs=4)
                nc.vector.reciprocal_approx_fast(out=csr, in_=csrow)
                cs_d = dram.tile([1, 512], FP, tag="cs_d", name="cs_d", bufs=4)
                nc.sync.dma_start(out=cs_d[:], in_=csr)
                # rec must share its base partition with dst (DVE 2-SBUF rule)
                rec = sbA.tile([128, 512], FP, tag="rec", name="rec", bufs=4)
                recs = rec[par * 64:par * 64 + DH, :]
                cs_d_ap = cs_d.opt()
                nc.gpsimd.dma_start(out=recs, in_=bass.AP(
                    tensor=cs_d_ap.tensor, offset=cs_d_ap.offset,
                    ap=[[0, DH]] + list(cs_d_ap.ap[1:])))
                nc.vector.tensor_mul(out=dst, in0=dstu, in1=recs)

        def outproj_rs_ln(s):
            sbB = pools["sbB"]
            # partial out-projection for this slice's 4 row blocks
            for it4 in range(4):
                it = s * 4 + it4
                op_sb = sbB.tile([128, D], FP, tag="op", name="op_sb")
                for nh in range(2):
                    op_ps = psP.tile([128, 512], FP, tag="ps", name="op_ps")
                    for kt in range(2):
                        nc.tensor.matmul(op_ps, OT[kt][:, it * 128:(it + 1) * 128],
                                         wo_sb[:, kt, nh * 512:(nh + 1) * 512],
                                         start=(kt == 0), stop=(kt == 1))
                    nc.vector.tensor_copy(out=op_sb[:, nh * 512:(nh + 1) * 512],
                                          in_=op_ps)
                nc.sync.dma_start(
                    out=oph[s][:].rearrange("(t p) d -> t p d", p=128)[it4],
                    in_=op_sb)
            nc.gpsimd.collective_compute(
                "ReduceScatter", mybir.AluOpType.add,
                replica_groups=[[0, 1, 2, 3], [4, 5, 6, 7]],
                ins=[oph[s].opt()], outs=[rsh[s].opt()])
            # residual + LayerNorm on this core's 128-row chunk
            xr = sbB.tile([128, D], FP, tag="xr", name="xr")
            rd = sbB.tile([128, D], FP, tag="rd", name="rd")
            rs_sb = sbB.tile([128, D], FP, tag="rsld", name="rs_sb")
            nc.sync.dma_start(out=rd, in_=res_tiles[s])
            nc.sync.dma_start(out=rs_sb, in_=rsh[s][:])
            nc.vector.tensor_add(out=xr, in0=rs_sb, in1=rd)
            stats = sbB.tile([128, 2, 6], FP, tag="stats", name="stats")
            mv = sbB.tile([128, 2], FP, tag="mv", name="mv")
            nc.vector.bn_stats(out=stats[:, 0, :], in_=xr[:, 0:512])
            nc.vector.bn_stats(out=stats[:, 1, :], in_=xr[:, 512:1024])
            nc.vector.bn_aggr(out=mv, in_=stats)
            # rstd = exp(-0.5*ln(var+eps)); Log/Exp share one ACT table set
            nc.scalar.activation(out=mv[:, 1:2], in_=mv[:, 1:2], func=AF.Ln,
                                 bias=eps_sb, scale=1.0)
            nc.scalar.activation(out=mv[:, 1:2], in_=mv[:, 1:2], func=AF.Exp,
                                 scale=-0.5)
            nc.vector.tensor_scalar(out=xr, in0=xr,
                                    scalar1=mv[:, 0:1], scalar2=mv[:, 1:2],
                                    op0=mybir.AluOpType.subtract,
                                    op1=mybir.AluOpType.mult)
            nc.vector.tensor_mul(out=xr, in0=xr, in1=g_sb)
            nc.vector.tensor_add(out=xr, in0=xr, in1=b_sb)
            nc.sync.dma_start(out=out_tiles[s], in_=xr)

        for s in range(NS):
            jobs = attention(s, 0)
            if s == 0:
                proj_qkp(1)  # overlaps first attention slice on other engines
            jobs += attention(s, 1)
            normalize(jobs)
            if s == 0:
                # x/pos/weight staging no longer needed; free its SBUF before
                # opening the out-proj/LN pool
                ph12_ctx.close()
                pools["sbB"] = ctx.enter_context(tc.tile_pool(name="sbB", bufs=2))
            outproj_rs_ln(s)

    nc.compile()
    return nc


_NC = None
_last_in_maps = None


def kernel(**inputs) -> np.ndarray:
    global _NC, _last_in_maps
    if _NC is None:
        _NC = build()
    nc = _NC

    q_s = np.asarray(inputs["q_s"], np.float32)
    pos = np.asarray(inputs["pos_emb"], np.float32)
    Wq = np.asarray(inputs["Wq"], np.float32)
    Wk = np.asarray(inputs["Wk"], np.float32)
    Wv = np.asarray(inputs["Wv"], np.float32)
    Wo = np.asarray(inputs["Wo"], np.float32)
    bo = np.asarray(inputs["bo"], np.float32)
    ln_g = np.asarray(inputs["ln_g"], np.float32)
    ln_b = np.asarray(inputs["ln_b"], np.float32)

    in_maps = []
    for c in range(NCORES):
        b, g = divmod(c, GRP)
        cs = slice(g * C, (g + 1) * C)
        resid = np.concatenate(
            [q_s[b][512 * s + 128 * g: 512 * s + 128 * (g + 1)] for s in range(NS)],
            axis=0) + bo[None, :]
        bf = ml_dtypes.bfloat16
        in_maps.append({
            "xT": np.ascontiguousarray(q_s[b].T.astype(bf)),
            "posT": np.ascontiguousarray(pos[b][:, cs].T),
            "wq": np.ascontiguousarray(Wq[:, cs].astype(bf)),
            "wk": np.ascontiguousarray(Wk[:, cs].astype(bf)),
            "wv": np.ascontiguousarray(Wv[:, cs].astype(bf)),
            "wo": np.ascontiguousarray(Wo[cs, :]),
            "resid": np.ascontiguousarray(resid),
            "ln_g": ln_g,
            "ln_b": ln_b,
        })

    _last_in_maps = in_maps
    res = run_bass_kernel_spmd(nc, in_maps, list(range(NCORES)))
    out = np.empty((B, N, D), np.float32)
    for c in range(NCORES):
        b, g = divmod(c, GRP)
        o = res.results[c]["out"]
        for s in range(NS):
            out[b, 512 * s + 128 * g: 512 * s + 128 * (g + 1), :] = \
                o[128 * s:128 * (s + 1)]
    return out



# revision 8
# speedup vs baseline: 1.1264x; 1.1264x over previous
"""BoT multi-head attention block (QKV proj + content/position attention +
out-proj + residual + LayerNorm) on 8 Trainium2 NeuronCores.

Sharding: tensor-parallel over heads (4 heads/core) x batch (2 batches, 4
cores each).  Each core computes q/k/v projections for its 256 feature
columns, full attention for its 4 heads, and a partial out-projection;
partials are summed with row-chunked ReduceScatters over each 4-core batch
group (overlapped with attention of later chunks), after which each core
applies residual + LayerNorm to its 4x128 rows.

Layout trick: attention logits are computed TRANSPOSED (j on partitions, i
free) so the softmax numerator matmul (P^T moving, V stationary) needs no
transpose of the probability matrix; an extra all-ones column in the
stationary V supplies the softmax denominator for free.  Host passes x and
pos pre-transposed.  Projections/out-proj run in fp32r, attention matmuls
in bf16 (1 cycle/col vs 2 for fp32r).
"""

import contextlib
import os
import sys

os.environ.setdefault("MYCRO_LOCAL_CACHE", "1")
for _p in ("/opt/trn_rl_repo",):
    if os.path.isdir(_p) and _p not in sys.path:
        sys.path.append(_p)

import ml_dtypes
import numpy as np

import concourse.bass as bass
from concourse import bacc
import concourse.mybir as mybir
import concourse.tile as tile
from concourse.bass_utils import run_bass_kernel_spmd

FP = mybir.dt.float32
FPR = mybir.dt.float32r
BF = mybir.dt.bfloat16
AF = mybir.ActivationFunctionType

B, N, D, H = 2, 2048, 1024, 16
NCORES = 8
GRP = 4                # cores per batch group
HPC = H // GRP         # heads per core = 4
C = D // GRP           # feature cols per core = 256
R = N // GRP           # output rows per core = 512
DH = D // H            # head dim = 64
SCALE = DH ** -0.5
LN_EPS = 1e-5

NT = N // 128          # 16 row tiles
KD = D // 128          # 8 contraction tiles over D
NS = N // 512          # 4 i-slices

ATT_DT = BF            # dtype of attention matmul operands
PROJ_DT = BF           # dtype of projection inputs (xT, wq/wk/wv)


def build():
    nc = bacc.Bacc("TRN2", target_bir_lowering=False, num_devices=NCORES)

    # fp32r-typed inputs are plain fp32 bits; typing them fp32r lets HWDGE
    # load them with no cast while satisfying the fp32r-producer rule.
    xT_t = nc.dram_tensor("xT", [D, N], PROJ_DT, kind="ExternalInput")
    posT_t = nc.dram_tensor("posT", [C, N], FP, kind="ExternalInput")
    wq_t = nc.dram_tensor("wq", [D, C], PROJ_DT, kind="ExternalInput")
    wk_t = nc.dram_tensor("wk", [D, C], PROJ_DT, kind="ExternalInput")
    wv_t = nc.dram_tensor("wv", [D, C], PROJ_DT, kind="ExternalInput")
    wo_t = nc.dram_tensor("wo", [C, D], FPR, kind="ExternalInput")
    res_t = nc.dram_tensor("resid", [R, D], FP, kind="ExternalInput")
    g_t = nc.dram_tensor("ln_g", [D], FP, kind="ExternalInput")
    bt_t = nc.dram_tensor("ln_b", [D], FP, kind="ExternalInput")
    out_t = nc.dram_tensor("out", [R, D], FP, kind="ExternalOutput")

    res_tiles = res_t.ap().rearrange("(t p) d -> t p d", p=128)
    out_tiles = out_t.ap().rearrange("(t p) d -> t p d", p=128)

    def bcast_ap(ap, parts):
        return bass.AP(tensor=ap.tensor, offset=ap.offset,
                       ap=[[0, parts]] + list(ap.ap))

    with tile.TileContext(nc) as tc, contextlib.ExitStack() as ctx:
        persist = ctx.enter_context(tc.tile_pool(name="persist", bufs=1))
        attnp = ctx.enter_context(tc.tile_pool(name="attnp", bufs=1))
        psP = ctx.enter_context(tc.tile_pool(name="psP", bufs=2, space="PSUM"))
        psO = ctx.enter_context(tc.tile_pool(name="psO", bufs=2, space="PSUM"))
        psC = ctx.enter_context(tc.tile_pool(name="psC", bufs=2, space="PSUM"))
        dram = ctx.enter_context(tc.tile_pool(name="dram", bufs=1, space="DRAM"))

        ones64 = persist.tile([1, DH], FP, tag="ones64")
        nc.vector.memset(ones64, 1.0)
        onescol = persist.tile([128, 1], FP, tag="onescol")
        nc.vector.memset(onescol, 1.0)

        sbA = ctx.enter_context(tc.tile_pool(name="sbA", bufs=3))

        # ---------------- phase 1-2: load (pre-transposed on host), project
        ph12_ctx = contextlib.ExitStack()
        p12 = ph12_ctx.enter_context(tc.tile_pool(name="ph12", bufs=1))

        wq_sb = p12.tile([128, KD, C], PROJ_DT, tag="wq")
        wk_sb = p12.tile([128, KD, C], PROJ_DT, tag="wk")
        wv_sb = p12.tile([128, KD, C], PROJ_DT, tag="wv")
        xT_sb = p12.tile([128, KD, N], PROJ_DT, tag="xT")
        xT_src = xT_t.ap().rearrange("(k p) n -> p k n", p=128)
        nc.sync.dma_start(out=wq_sb, in_=wq_t.ap().rearrange("(k p) c -> p k c", p=128))
        nc.sync.dma_start(out=wk_sb, in_=wk_t.ap().rearrange("(k p) c -> p k c", p=128))
        nc.sync.dma_start(out=wv_sb, in_=wv_t.ap().rearrange("(k p) c -> p k c", p=128))
        for k in range(KD):
            nc.sync.dma_start(out=xT_sb[:, k, :], in_=xT_src[:, k, :])
        xT = [xT_sb[:, k, :] for k in range(KD)]

        posT_sb = p12.tile([128, 2, N], FP, tag="posT")
        nc.sync.dma_start(out=posT_sb,
                          in_=posT_t.ap().rearrange("(m p) n -> p m n", p=128))
        posT = [posT_sb[:, m, :] for m in range(2)]

        # projections: qT/kpT [128 c, N] (head pair hp at rows 64*(h%2))
        qT = [attnp.tile([128, N], ATT_DT, name=f"qT{m}", tag=f"qT{m}") for m in range(2)]
        kpT = [attnp.tile([128, N], ATT_DT, name=f"kpT{m}", tag=f"kpT{m}") for m in range(2)]
        V = [attnp.tile([128, HPC, DH + 1], ATT_DT, name=f"V{t}", tag=f"V{t}")
             for t in range(NT)]

        def proj_qkp(m):
            for s in range(NS):
                q_ps = psP.tile([128, 512], FP, tag="ps", name="q_ps")
                for k in range(KD):
                    nc.tensor.matmul(q_ps, wq_sb[:, k, m * 128:(m + 1) * 128],
                                     xT[k][:, s * 512:(s + 1) * 512],
                                     start=(k == 0), stop=(k == KD - 1))
                nc.vector.tensor_copy(out=qT[m][:, s * 512:(s + 1) * 512], in_=q_ps)
            for s in range(NS):
                kp_ps = psP.tile([128, 512], FP, tag="ps", name="kp_ps")
                for k in range(KD):
                    nc.tensor.matmul(kp_ps, wk_sb[:, k, m * 128:(m + 1) * 128],
                                     xT[k][:, s * 512:(s + 1) * 512],
                                     start=(k == 0), stop=(k == KD - 1))
                nc.vector.tensor_add(out=kpT[m][:, s * 512:(s + 1) * 512],
                                     in0=kp_ps, in1=posT[m][:, s * 512:(s + 1) * 512])

        proj_qkp(0)
        for t in range(NT):
            v_ps = psP.tile([128, C], FP, tag="ps", name="v_ps")
            for k in range(KD):
                nc.tensor.matmul(v_ps, xT[k][:, t * 128:(t + 1) * 128], wv_sb[:, k, :],
                                 start=(k == 0), stop=(k == KD - 1))
            nc.vector.tensor_copy(out=V[t][:, :, 0:DH],
                                  in_=v_ps.rearrange("p (h d) -> p h d", h=HPC))
            nc.vector.tensor_copy(out=V[t][:, :, DH:DH + 1],
                                  in_=onescol.broadcast_to([128, HPC, 1]))

        # ---------------- phases 3-4 interleaved per i-slice s -------------
        pools = {}

        wo_sb = persist.tile([128, 2, D], FPR, tag="wo")
        nc.sync.dma_start(out=wo_sb, in_=wo_t.ap().rearrange("(k p) d -> p k d", p=128))
        g_sb = persist.tile([128, D], FP, tag="g")
        b_sb = persist.tile([128, D], FP, tag="b")
        nc.gpsimd.dma_start(out=g_sb, in_=bcast_ap(g_t.ap(), 128))
        nc.gpsimd.dma_start(out=b_sb, in_=bcast_ap(bt_t.ap(), 128))
        eps_sb = persist.tile([128, 1], FP, tag="eps")
        nc.vector.memset(eps_sb, LN_EPS)

        OT = [attnp.tile([128, N], FPR, name=f"OT{m}", tag=f"OT{m}") for m in range(2)]
        OTU = [attnp.tile([128, N], FP, name=f"OTU{m}", tag=f"OTU{m}") for m in range(2)]
        oph = [dram.tile([R, D], BF, name=f"oph{s}", tag=f"oph{s}") for s in range(NS)]
        rsh = [dram.tile([128, D], BF, name=f"rsh{s}", tag=f"rsh{s}")
               for s in range(NS)]

        def attention(s, hp):
            ot_e = psO.tile([128, 512], FP, tag="ot", name="ot_e")
            ot_o = psO.tile([128, 512], FP, tag="ot", name="ot_o")
            for jt in range(NT):
                st = psC.tile([128, 1024], FP, tag="st", name="st")
                nc.tensor.matmul(st[:, 0:512],
                                 kpT[hp][0:64, jt * 128:(jt + 1) * 128],
                                 qT[hp][0:64, s * 512:(s + 1) * 512],
                                 start=True, stop=True)
                nc.tensor.matmul(st[:, 512:1024],
                                 kpT[hp][64:128, jt * 128:(jt + 1) * 128],
                                 qT[hp][64:128, s * 512:(s + 1) * 512],
                                 start=True, stop=True)
                ste = sbA.tile([128, 1024], ATT_DT, tag="ste", name="ste")
                nc.scalar.activation(out=ste, in_=st, func=AF.Exp, scale=SCALE)
                nc.tensor.matmul(ot_e[0:DH + 1, :], V[jt][:, 2 * hp, :],
                                 ste[:, 0:512],
                                 start=(jt == 0), stop=(jt == NT - 1))
                nc.tensor.matmul(ot_o[0:DH + 1, :], V[jt][:, 2 * hp + 1, :],
                                 ste[:, 512:1024],
                                 start=(jt == 0), stop=(jt == NT - 1))
            # evacuate PSUM immediately: unnormalized OT rows + colsum row to
            # SBUF (releases the ot accumulators within ~1.5us); the actual
            # softmax division happens later, overlapped with the next slice
            jobs = []
            for par, ot in ((0, ot_e), (1, ot_o)):
                csrow = sbA.tile([1, 512], FP, tag="csrow", name="csrow", bufs=8)
                nc.vector.tensor_copy(out=csrow, in_=ot[DH:DH + 1, :])
                dst = OT[hp][par * 64:par * 64 + DH, s * 512:(s + 1) * 512]
                dstu = OTU[hp][par * 64:par * 64 + DH, s * 512:(s + 1) * 512]
                nc.vector.tensor_copy(out=dstu, in_=ot[0:DH, :])
                jobs.append((dst, dstu, csrow, par))
            return jobs

        def normalize(jobs):
            for dst, dstu, csrow, par in jobs:
                csr = sbA.tile([1, 512], FP, tag="csr", name="csr", bufs=4)
                nc.vector.reciprocal_approx_fast(out=csr, in_=csrow)
                cs_d = dram.tile([1, 512], FP, tag="cs_d", name="cs_d", bufs=4)
                nc.sync.dma_start(out=cs_d[:], in_=csr)
                # rec must share its base partition with dst (DVE 2-SBUF rule)
                rec = sbA.tile([128, 512], FP, tag="rec", name="rec", bufs=4)
                recs = rec[par * 64:par * 64 + DH, :]
                cs_d_ap = cs_d.opt()
                # broadcast on the SP queue: gpsimd is the collective queue,
                # and routing this DMA there serializes each slice's softmax
                # normalization behind the previous slice's ReduceScatter
                nc.sync.dma_start(out=recs, in_=bass.AP(
                    tensor=cs_d_ap.tensor, offset=cs_d_ap.offset,
                    ap=[[0, DH]] + list(cs_d_ap.ap[1:])))
                nc.vector.tensor_mul(out=dst, in0=dstu, in1=recs)

        def outproj_rs_ln(s):
            sbB = pools["sbB"]
            # partial out-projection for this slice's 4 row blocks
            for it4 in range(4):
                it = s * 4 + it4
                op_sb = sbB.tile([128, D], BF, tag="op", name="op_sb")
                for nh in range(2):
                    op_ps = psP.tile([128, 512], FP, tag="ps", name="op_ps")
                    for kt in range(2):
                        nc.tensor.matmul(op_ps, OT[kt][:, it * 128:(it + 1) * 128],
                                         wo_sb[:, kt, nh * 512:(nh + 1) * 512],
                                         start=(kt == 0), stop=(kt == 1))
                    nc.vector.tensor_copy(out=op_sb[:, nh * 512:(nh + 1) * 512],
                                          in_=op_ps)
                nc.sync.dma_start(
                    out=oph[s][:].rearrange("(t p) d -> t p d", p=128)[it4],
                    in_=op_sb)
            nc.gpsimd.collective_compute(
                "ReduceScatter", mybir.AluOpType.add,
                replica_groups=[[0, 1, 2, 3], [4, 5, 6, 7]],
                ins=[oph[s].opt()], outs=[rsh[s].opt()])
            # residual + LayerNorm on this core's 128-row chunk
            xr = sbB.tile([128, D], FP, tag="xr", name="xr")
            rd = sbB.tile([128, D], FP, tag="rd", name="rd")
            rs_sb = sbB.tile([128, D], BF, tag="rsld", name="rs_sb")
            nc.sync.dma_start(out=rd, in_=res_tiles[s])
            nc.sync.dma_start(out=rs_sb, in_=rsh[s][:])
            nc.vector.tensor_add(out=xr, in0=rs_sb, in1=rd)
            stats = sbB.tile([128, 2, 6], FP, tag="stats", name="stats")
            mv = sbB.tile([128, 2], FP, tag="mv", name="mv")
            nc.vector.bn_stats(out=stats[:, 0, :], in_=xr[:, 0:512])
            nc.vector.bn_stats(out=stats[:, 1, :], in_=xr[:, 512:1024])
            nc.vector.bn_aggr(out=mv, in_=stats)
            # rstd = exp(-0.5*ln(var+eps)); Log/Exp share one ACT table set
            nc.scalar.activation(out=mv[:, 1:2], in_=mv[:, 1:2], func=AF.Ln,
                                 bias=eps_sb, scale=1.0)
            nc.scalar.activation(out=mv[:, 1:2], in_=mv[:, 1:2], func=AF.Exp,
                                 scale=-0.5)
            nc.vector.tensor_scalar(out=xr, in0=xr,
                                    scalar1=mv[:, 0:1], scalar2=mv[:, 1:2],
                                    op0=mybir.AluOpType.subtract,
                                    op1=mybir.AluOpType.mult)
            nc.vector.tensor_mul(out=xr, in0=xr, in1=g_sb)
            nc.vector.tensor_add(out=xr, in0=xr, in1=b_sb)
            nc.sync.dma_start(out=out_tiles[s], in_=xr)

        for s in range(NS):
            jobs = attention(s, 0)
            if s == 0:
                proj_qkp(1)  # overlaps first attention slice on other engines
            jobs += attention(s, 1)
            normalize(jobs)
            if s == 0:
                # x/pos/weight staging no longer needed; free its SBUF before
                # opening the out-proj/LN pool
                ph12_ctx.close()
                pools["sbB"] = ctx.enter_context(tc.tile_pool(name="sbB", bufs=2))
            outproj_rs_ln(s)

    nc.compile()
    return nc


_NC = None
_last_in_maps = None


def kernel(**inputs) -> np.ndarray:
    global _NC, _last_in_maps
    if _NC is None:
        _NC = build()
    nc = _NC

    q_s = np.asarray(inputs["q_s"], np.float32)
    pos = np.asarray(inputs["pos_emb"], np.float32)
    Wq = np.asarray(inputs["Wq"], np.float32)
    Wk = np.asarray(inputs["Wk"], np.float32)
    Wv = np.asarray(inputs["Wv"], np.float32)
    Wo = np.asarray(inputs["Wo"], np.float32)
    bo = np.asarray(inputs["bo"], np.float32)
    ln_g = np.asarray(inputs["ln_g"], np.float32)
    ln_b = np.asarray(inputs["ln_b"], np.float32)

    in_maps = []
    for c in range(NCORES):
        b, g = divmod(c, GRP)
        cs = slice(g * C, (g + 1) * C)
        resid = np.concatenate(
            [q_s[b][512 * s + 128 * g: 512 * s + 128 * (g + 1)] for s in range(NS)],
            axis=0) + bo[None, :]
        bf = ml_dtypes.bfloat16
        in_maps.append({
            "xT": np.ascontiguousarray(q_s[b].T.astype(bf)),
            "posT": np.ascontiguousarray(pos[b][:, cs].T),
            "wq": np.ascontiguousarray(Wq[:, cs].astype(bf)),
            "wk": np.ascontiguousarray(Wk[:, cs].astype(bf)),
            "wv": np.ascontiguousarray(Wv[:, cs].astype(bf)),
            "wo": np.ascontiguousarray(Wo[cs, :]),
            "resid": np.ascontiguousarray(resid),
            "ln_g": ln_g,
            "ln_b": ln_b,
        })

    _last_in_maps = in_maps
    res = run_bass_kernel_spmd(nc, in_maps, list(range(NCORES)))
    out = np.empty((B, N, D), np.float32)
    for c in range(NCORES):
        b, g = divmod(c, GRP)
        o = res.results[c]["out"]
        for s in range(NS):
            out[b, 512 * s + 128 * g: 512 * s + 128 * (g + 1), :] = \
                o[128 * s:128 * (s + 1)]
    return out



# revision 12
# speedup vs baseline: 1.1456x; 1.0170x over previous
"""BoT multi-head attention block (QKV proj + content/position attention +
out-proj + residual + LayerNorm) on 8 Trainium2 NeuronCores.

Sharding: tensor-parallel over heads (4 heads/core) x batch (2 batches, 4
cores each).  Each core computes q/k/v projections for its 256 feature
columns, full attention for its 4 heads, and a partial out-projection;
partials are summed with row-chunked ReduceScatters over each 4-core batch
group (overlapped with attention of later chunks), after which each core
applies residual + LayerNorm to its 4x128 rows.

Layout trick: attention logits are computed TRANSPOSED (j on partitions, i
free) so the softmax numerator matmul (P^T moving, V stationary) needs no
transpose of the probability matrix; an extra all-ones column in the
stationary V supplies the softmax denominator for free.  Host passes x and
pos pre-transposed.  Projections/out-proj run in fp32r, attention matmuls
in bf16 (1 cycle/col vs 2 for fp32r).
"""

import contextlib
import os
import sys

os.environ.setdefault("MYCRO_LOCAL_CACHE", "1")
for _p in ("/opt/trn_rl_repo",):
    if os.path.isdir(_p) and _p not in sys.path:
        sys.path.append(_p)

import ml_dtypes
import numpy as np

import concourse.bass as bass
from concourse import bacc
import concourse.mybir as mybir
import concourse.tile as tile
from concourse.bass_utils import run_bass_kernel_spmd

FP = mybir.dt.float32
FPR = mybir.dt.float32r
BF = mybir.dt.bfloat16
AF = mybir.ActivationFunctionType

B, N, D, H = 2, 2048, 1024, 16
NCORES = 8
GRP = 4                # cores per batch group
HPC = H // GRP         # heads per core = 4
C = D // GRP           # feature cols per core = 256
R = N // GRP           # output rows per core = 512
DH = D // H            # head dim = 64
SCALE = DH ** -0.5
LN_EPS = 1e-5

NT = N // 128          # 16 row tiles
KD = D // 128          # 8 contraction tiles over D
NS = N // 512          # 4 i-slices

ATT_DT = BF            # dtype of attention matmul operands
PROJ_DT = BF           # dtype of projection inputs (xT, wq/wk/wv)


def build():
    nc = bacc.Bacc("TRN2", target_bir_lowering=False, num_devices=NCORES)

    # fp32r-typed inputs are plain fp32 bits; typing them fp32r lets HWDGE
    # load them with no cast while satisfying the fp32r-producer rule.
    xT_t = nc.dram_tensor("xT", [D, N], PROJ_DT, kind="ExternalInput")
    posT_t = nc.dram_tensor("posT", [C, N], BF, kind="ExternalInput")
    wq_t = nc.dram_tensor("wq", [D, C], PROJ_DT, kind="ExternalInput")
    wk_t = nc.dram_tensor("wk", [D, C], PROJ_DT, kind="ExternalInput")
    wv_t = nc.dram_tensor("wv", [D, C], PROJ_DT, kind="ExternalInput")
    wo_t = nc.dram_tensor("wo", [C, D], FPR, kind="ExternalInput")
    res_t = nc.dram_tensor("resid", [R, D], FP, kind="ExternalInput")
    g_t = nc.dram_tensor("ln_g", [D], FP, kind="ExternalInput")
    bt_t = nc.dram_tensor("ln_b", [D], FP, kind="ExternalInput")
    out_t = nc.dram_tensor("out", [R, D], FP, kind="ExternalOutput")

    res_tiles = res_t.ap().rearrange("(t p) d -> t p d", p=128)
    out_tiles = out_t.ap().rearrange("(t p) d -> t p d", p=128)

    def bcast_ap(ap, parts):
        return bass.AP(tensor=ap.tensor, offset=ap.offset,
                       ap=[[0, parts]] + list(ap.ap))

    with tile.TileContext(nc) as tc, contextlib.ExitStack() as ctx:
        persist = ctx.enter_context(tc.tile_pool(name="persist", bufs=1))
        attnp = ctx.enter_context(tc.tile_pool(name="attnp", bufs=1))
        psP = ctx.enter_context(tc.tile_pool(name="psP", bufs=2, space="PSUM"))
        psO = ctx.enter_context(tc.tile_pool(name="psO", bufs=2, space="PSUM"))
        psC = ctx.enter_context(tc.tile_pool(name="psC", bufs=2, space="PSUM"))
        dram = ctx.enter_context(tc.tile_pool(name="dram", bufs=1, space="DRAM"))

        ones64 = persist.tile([1, DH], FP, tag="ones64")
        nc.vector.memset(ones64, 1.0)
        onescol = persist.tile([128, 1], FP, tag="onescol")
        nc.vector.memset(onescol, 1.0)

        # tiny warmup collective: absorbs one-time CC ring setup (~40us on
        # the first collective) while the input DMAs stream in
        ccw_sb = persist.tile([4, 4], BF, tag="ccw_sb")
        nc.vector.memset(ccw_sb, 0.0)
        ccw_in = dram.tile([4, 4], BF, name="ccw_in", tag="ccw_in")
        ccw_out = dram.tile([1, 4], BF, name="ccw_out", tag="ccw_out")
        nc.sync.dma_start(out=ccw_in[:], in_=ccw_sb)
        nc.gpsimd.collective_compute(
            "ReduceScatter", mybir.AluOpType.add,
            replica_groups=[[0, 1, 2, 3], [4, 5, 6, 7]],
            ins=[ccw_in.opt()], outs=[ccw_out.opt()])

        sbA = ctx.enter_context(tc.tile_pool(name="sbA", bufs=3))

        # ---------------- phase 1-2: load (pre-transposed on host), project
        ph12_ctx = contextlib.ExitStack()
        p12 = ph12_ctx.enter_context(tc.tile_pool(name="ph12", bufs=1))

        wq_sb = p12.tile([128, KD, C], PROJ_DT, tag="wq")
        wk_sb = p12.tile([128, KD, C], PROJ_DT, tag="wk")
        wv_sb = p12.tile([128, KD, C], PROJ_DT, tag="wv")
        xT_sb = p12.tile([128, KD, N], PROJ_DT, tag="xT")
        xT_src = xT_t.ap().rearrange("(k p) n -> p k n", p=128)
        nc.sync.dma_start(out=wq_sb, in_=wq_t.ap().rearrange("(k p) c -> p k c", p=128))
        nc.sync.dma_start(out=wk_sb, in_=wk_t.ap().rearrange("(k p) c -> p k c", p=128))
        nc.sync.dma_start(out=wv_sb, in_=wv_t.ap().rearrange("(k p) c -> p k c", p=128))
        for k in range(KD):
            nc.sync.dma_start(out=xT_sb[:, k, :], in_=xT_src[:, k, :])
        xT = [xT_sb[:, k, :] for k in range(KD)]

        posT_sb = p12.tile([128, 2, N], BF, tag="posT")
        nc.sync.dma_start(out=posT_sb,
                          in_=posT_t.ap().rearrange("(m p) n -> p m n", p=128))
        posT = [posT_sb[:, m, :] for m in range(2)]

        # projections: qT/kpT [128 c, N] (head pair hp at rows 64*(h%2))
        qT = [attnp.tile([128, N], ATT_DT, name=f"qT{m}", tag=f"qT{m}") for m in range(2)]
        kpT = [attnp.tile([128, N], ATT_DT, name=f"kpT{m}", tag=f"kpT{m}") for m in range(2)]
        V = [attnp.tile([128, HPC, DH + 1], ATT_DT, name=f"V{t}", tag=f"V{t}")
             for t in range(NT)]

        def proj_qkp(m):
            for s in range(NS):
                q_ps = psP.tile([128, 512], FP, tag="ps", name="q_ps")
                for k in range(KD):
                    nc.tensor.matmul(q_ps, wq_sb[:, k, m * 128:(m + 1) * 128],
                                     xT[k][:, s * 512:(s + 1) * 512],
                                     start=(k == 0), stop=(k == KD - 1))
                nc.vector.tensor_copy(out=qT[m][:, s * 512:(s + 1) * 512], in_=q_ps)
            for s in range(NS):
                kp_ps = psP.tile([128, 512], FP, tag="ps", name="kp_ps")
                for k in range(KD):
                    nc.tensor.matmul(kp_ps, wk_sb[:, k, m * 128:(m + 1) * 128],
                                     xT[k][:, s * 512:(s + 1) * 512],
                                     start=(k == 0), stop=(k == KD - 1))
                nc.vector.tensor_add(out=kpT[m][:, s * 512:(s + 1) * 512],
                                     in0=kp_ps, in1=posT[m][:, s * 512:(s + 1) * 512])

        proj_qkp(0)
        for t in range(NT):
            v_ps = psP.tile([128, C], FP, tag="ps", name="v_ps")
            for k in range(KD):
                nc.tensor.matmul(v_ps, xT[k][:, t * 128:(t + 1) * 128], wv_sb[:, k, :],
                                 start=(k == 0), stop=(k == KD - 1))
            nc.vector.tensor_copy(out=V[t][:, :, 0:DH],
                                  in_=v_ps.rearrange("p (h d) -> p h d", h=HPC))
            nc.vector.tensor_copy(out=V[t][:, :, DH:DH + 1],
                                  in_=onescol.broadcast_to([128, HPC, 1]))

        # ---------------- phases 3-4 interleaved per i-slice s -------------
        pools = {}

        wo_sb = persist.tile([128, 2, D], FPR, tag="wo")
        nc.sync.dma_start(out=wo_sb, in_=wo_t.ap().rearrange("(k p) d -> p k d", p=128))
        g_sb = persist.tile([128, D], FP, tag="g")
        b_sb = persist.tile([128, D], FP, tag="b")
        nc.gpsimd.dma_start(out=g_sb, in_=bcast_ap(g_t.ap(), 128))
        nc.gpsimd.dma_start(out=b_sb, in_=bcast_ap(bt_t.ap(), 128))
        eps_sb = persist.tile([128, 1], FP, tag="eps")
        nc.vector.memset(eps_sb, LN_EPS)

        OT = [attnp.tile([128, N], FPR, name=f"OT{m}", tag=f"OT{m}") for m in range(2)]
        OTU = [attnp.tile([128, N], FP, name=f"OTU{m}", tag=f"OTU{m}") for m in range(2)]
        oph = [dram.tile([R, D], BF, name=f"oph{s}", tag=f"oph{s}") for s in range(NS)]
        rsh = [dram.tile([128, D], BF, name=f"rsh{s}", tag=f"rsh{s}")
               for s in range(NS)]

        def attention(s, hp):
            ot_e = psO.tile([128, 512], FP, tag="ot", name="ot_e")
            ot_o = psO.tile([128, 512], FP, tag="ot", name="ot_o")
            for jt in range(NT):
                st = psC.tile([128, 1024], FP, tag="st", name="st")
                nc.tensor.matmul(st[:, 0:512],
                                 kpT[hp][0:64, jt * 128:(jt + 1) * 128],
                                 qT[hp][0:64, s * 512:(s + 1) * 512],
                                 start=True, stop=True)
                nc.tensor.matmul(st[:, 512:1024],
                                 kpT[hp][64:128, jt * 128:(jt + 1) * 128],
                                 qT[hp][64:128, s * 512:(s + 1) * 512],
                                 start=True, stop=True)
                ste = sbA.tile([128, 1024], ATT_DT, tag="ste", name="ste")
                nc.scalar.activation(out=ste, in_=st, func=AF.Exp, scale=SCALE)
                nc.tensor.matmul(ot_e[0:DH + 1, :], V[jt][:, 2 * hp, :],
                                 ste[:, 0:512],
                                 start=(jt == 0), stop=(jt == NT - 1))
                nc.tensor.matmul(ot_o[0:DH + 1, :], V[jt][:, 2 * hp + 1, :],
                                 ste[:, 512:1024],
                                 start=(jt == 0), stop=(jt == NT - 1))
            # evacuate PSUM immediately: unnormalized OT rows + colsum row to
            # SBUF (releases the ot accumulators within ~1.5us); the actual
            # softmax division happens later, overlapped with the next slice
            jobs = []
            for par, ot in ((0, ot_e), (1, ot_o)):
                csrow = sbA.tile([1, 512], FP, tag="csrow", name="csrow", bufs=8)
                nc.vector.tensor_copy(out=csrow, in_=ot[DH:DH + 1, :])
                dst = OT[hp][par * 64:par * 64 + DH, s * 512:(s + 1) * 512]
                dstu = OTU[hp][par * 64:par * 64 + DH, s * 512:(s + 1) * 512]
                nc.vector.tensor_copy(out=dstu, in_=ot[0:DH, :])
                jobs.append((dst, dstu, csrow, par))
            return jobs

        def normalize(jobs):
            for dst, dstu, csrow, par in jobs:
                csr = sbA.tile([1, 512], FP, tag="csr", name="csr", bufs=4)
                nc.vector.reciprocal_approx_fast(out=csr, in_=csrow)
                cs_d = dram.tile([1, 512], FP, tag="cs_d", name="cs_d", bufs=4)
                nc.sync.dma_start(out=cs_d[:], in_=csr)
                # rec must share its base partition with dst (DVE 2-SBUF rule)
                rec = sbA.tile([128, 512], FP, tag="rec", name="rec", bufs=4)
                recs = rec[par * 64:par * 64 + DH, :]
                cs_d_ap = cs_d.opt()
                # broadcast on the SP queue: gpsimd is the collective queue,
                # and routing this DMA there serializes each slice's softmax
                # normalization behind the previous slice's ReduceScatter
                nc.sync.dma_start(out=recs, in_=bass.AP(
                    tensor=cs_d_ap.tensor, offset=cs_d_ap.offset,
                    ap=[[0, DH]] + list(cs_d_ap.ap[1:])))
                nc.vector.tensor_mul(out=dst, in0=dstu, in1=recs)

        def outproj_rs_ln(s):
            sbB = pools["sbB"]
            # partial out-projection for this slice's 4 row blocks
            for it4 in range(4):
                it = s * 4 + it4
                op_sb = sbB.tile([128, D], BF, tag="op", name="op_sb")
                for nh in range(2):
                    op_ps = psP.tile([128, 512], FP, tag="ps", name="op_ps")
                    for kt in range(2):
                        nc.tensor.matmul(op_ps, OT[kt][:, it * 128:(it + 1) * 128],
                                         wo_sb[:, kt, nh * 512:(nh + 1) * 512],
                                         start=(kt == 0), stop=(kt == 1))
                    nc.vector.tensor_copy(out=op_sb[:, nh * 512:(nh + 1) * 512],
                                          in_=op_ps)
                nc.sync.dma_start(
                    out=oph[s][:].rearrange("(t p) d -> t p d", p=128)[it4],
                    in_=op_sb)
            nc.gpsimd.collective_compute(
                "ReduceScatter", mybir.AluOpType.add,
                replica_groups=[[0, 1, 2, 3], [4, 5, 6, 7]],
                ins=[oph[s].opt()], outs=[rsh[s].opt()])

        def ln_tail(s):
            # residual + LayerNorm on this core's 128-row chunk; deferred to
            # the tail so no engine queue ever blocks on a ReduceScatter
            # mid-pipeline (in-order queues propagate such a wait everywhere)
            sbB = pools["sbB"]
            xr = sbB.tile([128, D], FP, tag="xr", name="xr")
            rd = sbB.tile([128, D], FP, tag="rd", name="rd")
            rs_sb = sbB.tile([128, D], BF, tag="rsld", name="rs_sb")
            nc.sync.dma_start(out=rd, in_=res_tiles[s])
            nc.sync.dma_start(out=rs_sb, in_=rsh[s][:])
            nc.vector.tensor_add(out=xr, in0=rs_sb, in1=rd)
            stats = sbB.tile([128, 2, 6], FP, tag="stats", name="stats")
            mv = sbB.tile([128, 2], FP, tag="mv", name="mv")
            nc.vector.bn_stats(out=stats[:, 0, :], in_=xr[:, 0:512])
            nc.vector.bn_stats(out=stats[:, 1, :], in_=xr[:, 512:1024])
            nc.vector.bn_aggr(out=mv, in_=stats)
            # rstd = exp(-0.5*ln(var+eps)); Log/Exp share one ACT table set
            nc.scalar.activation(out=mv[:, 1:2], in_=mv[:, 1:2], func=AF.Ln,
                                 bias=eps_sb, scale=1.0)
            nc.scalar.activation(out=mv[:, 1:2], in_=mv[:, 1:2], func=AF.Exp,
                                 scale=-0.5)
            nc.vector.tensor_scalar(out=xr, in0=xr,
                                    scalar1=mv[:, 0:1], scalar2=mv[:, 1:2],
                                    op0=mybir.AluOpType.subtract,
                                    op1=mybir.AluOpType.mult)
            nc.vector.tensor_mul(out=xr, in0=xr, in1=g_sb)
            nc.vector.tensor_add(out=xr, in0=xr, in1=b_sb)
            nc.sync.dma_start(out=out_tiles[s], in_=xr)

        for s in range(NS):
            jobs = attention(s, 0)
            if s == 0:
                proj_qkp(1)  # overlaps first attention slice on other engines
            jobs += attention(s, 1)
            normalize(jobs)
            if s == 0:
                # x/pos/weight staging no longer needed; free its SBUF before
                # opening the out-proj/LN pool
                ph12_ctx.close()
                pools["sbB"] = ctx.enter_context(tc.tile_pool(name="sbB", bufs=2))
            outproj_rs_ln(s)
        for s in range(NS):
            ln_tail(s)

    nc.compile()
    return nc


_NC = None
_last_in_maps = None


def kernel(**inputs) -> np.ndarray:
    global _NC, _last_in_maps
    if _NC is None:
        _NC = build()
    nc = _NC

    q_s = np.asarray(inputs["q_s"], np.float32)
    pos = np.asarray(inputs["pos_emb"], np.float32)
    Wq = np.asarray(inputs["Wq"], np.float32)
    Wk = np.asarray(inputs["Wk"], np.float32)
    Wv = np.asarray(inputs["Wv"], np.float32)
    Wo = np.asarray(inputs["Wo"], np.float32)
    bo = np.asarray(inputs["bo"], np.float32)
    ln_g = np.asarray(inputs["ln_g"], np.float32)
    ln_b = np.asarray(inputs["ln_b"], np.float32)

    in_maps = []
    for c in range(NCORES):
        b, g = divmod(c, GRP)
        cs = slice(g * C, (g + 1) * C)
        resid = np.concatenate(
            [q_s[b][512 * s + 128 * g: 512 * s + 128 * (g + 1)] for s in range(NS)],
            axis=0) + bo[None, :]
        bf = ml_dtypes.bfloat16
        in_maps.append({
            "xT": np.ascontiguousarray(q_s[b].T.astype(bf)),
            "posT": np.ascontiguousarray(pos[b][:, cs].T.astype(bf)),
            "wq": np.ascontiguousarray(Wq[:, cs].astype(bf)),
            "wk": np.ascontiguousarray(Wk[:, cs].astype(bf)),
            "wv": np.ascontiguousarray(Wv[:, cs].astype(bf)),
            "wo": np.ascontiguousarray(Wo[cs, :]),
            "resid": np.ascontiguousarray(resid),
            "ln_g": ln_g,
            "ln_b": ln_b,
        })

    _last_in_maps = in_maps
    res = run_bass_kernel_spmd(nc, in_maps, list(range(NCORES)))
    out = np.empty((B, N, D), np.float32)
    for c in range(NCORES):
        b, g = divmod(c, GRP)
        o = res.results[c]["out"]
        for s in range(NS):
            out[b, 512 * s + 128 * g: 512 * s + 128 * (g + 1), :] = \
                o[128 * s:128 * (s + 1)]
    return out



# revision 14
# speedup vs baseline: 1.2063x; 1.0530x over previous
"""BoT multi-head attention block (QKV proj + content/position attention +
out-proj + residual + LayerNorm) on 8 Trainium2 NeuronCores.

Sharding: tensor-parallel over heads (4 heads/core) x batch (2 batches, 4
cores each).  Each core computes q/k/v projections for its 256 feature
columns, full attention for its 4 heads, and a partial out-projection;
partials are summed with row-chunked ReduceScatters over each 4-core batch
group (overlapped with attention of later chunks), after which each core
applies residual + LayerNorm to its 4x128 rows.

Layout trick: attention logits are computed TRANSPOSED (j on partitions, i
free) so the softmax numerator matmul (P^T moving, V stationary) needs no
transpose of the probability matrix; an extra all-ones column in the
stationary V supplies the softmax denominator for free.  Host passes x and
pos pre-transposed.  Projections/out-proj run in fp32r, attention matmuls
in bf16 (1 cycle/col vs 2 for fp32r).
"""

import contextlib
import os
import sys

os.environ.setdefault("MYCRO_LOCAL_CACHE", "1")
for _p in ("/opt/trn_rl_repo",):
    if os.path.isdir(_p) and _p not in sys.path:
        sys.path.append(_p)

import ml_dtypes
import numpy as np

import concourse.bass as bass
from concourse import bacc
import concourse.mybir as mybir
import concourse.tile as tile
from concourse.bass_utils import run_bass_kernel_spmd

FP = mybir.dt.float32
FPR = mybir.dt.float32r
BF = mybir.dt.bfloat16
AF = mybir.ActivationFunctionType

B, N, D, H = 2, 2048, 1024, 16
NCORES = 8
GRP = 4                # cores per batch group
HPC = H // GRP         # heads per core = 4
C = D // GRP           # feature cols per core = 256
R = N // GRP           # output rows per core = 512
DH = D // H            # head dim = 64
SCALE = DH ** -0.5
LN_EPS = 1e-5

NT = N // 128          # 16 row tiles
KD = D // 128          # 8 contraction tiles over D
NS = N // 512          # 4 i-slices

ATT_DT = BF            # dtype of attention matmul operands
PROJ_DT = BF           # dtype of projection inputs (xT, wq/wk/wv)


def build():
    nc = bacc.Bacc("TRN2", target_bir_lowering=False, num_devices=NCORES)

    # fp32r-typed inputs are plain fp32 bits; typing them fp32r lets HWDGE
    # load them with no cast while satisfying the fp32r-producer rule.
    xT_t = nc.dram_tensor("xT", [D, N], PROJ_DT, kind="ExternalInput")
    posT_t = nc.dram_tensor("posT", [C, N], BF, kind="ExternalInput")
    wq_t = nc.dram_tensor("wq", [D, C], PROJ_DT, kind="ExternalInput")
    wk_t = nc.dram_tensor("wk", [D, C], PROJ_DT, kind="ExternalInput")
    wv_t = nc.dram_tensor("wv", [D, C], PROJ_DT, kind="ExternalInput")
    wo_t = nc.dram_tensor("wo", [C, D], FPR, kind="ExternalInput")
    res_t = nc.dram_tensor("resid", [R, D], FP, kind="ExternalInput")
    g_t = nc.dram_tensor("ln_g", [D], FP, kind="ExternalInput")
    bt_t = nc.dram_tensor("ln_b", [D], FP, kind="ExternalInput")
    out_t = nc.dram_tensor("out", [R, D], FP, kind="ExternalOutput")

    res_tiles = res_t.ap().rearrange("(t p) d -> t p d", p=128)
    out_tiles = out_t.ap().rearrange("(t p) d -> t p d", p=128)

    def bcast_ap(ap, parts):
        return bass.AP(tensor=ap.tensor, offset=ap.offset,
                       ap=[[0, parts]] + list(ap.ap))

    with tile.TileContext(nc) as tc, contextlib.ExitStack() as ctx:
        persist = ctx.enter_context(tc.tile_pool(name="persist", bufs=1))
        attnp = ctx.enter_context(tc.tile_pool(name="attnp", bufs=1))
        psP = ctx.enter_context(tc.tile_pool(name="psP", bufs=2, space="PSUM"))
        psO = ctx.enter_context(tc.tile_pool(name="psO", bufs=2, space="PSUM"))
        psC = ctx.enter_context(tc.tile_pool(name="psC", bufs=2, space="PSUM"))
        dram = ctx.enter_context(tc.tile_pool(name="dram", bufs=1, space="DRAM"))

        ones64 = persist.tile([1, DH], FP, tag="ones64")
        nc.vector.memset(ones64, 1.0)
        onescol = persist.tile([128, 1], FP, tag="onescol")
        nc.vector.memset(onescol, 1.0)

        # tiny warmup collective: absorbs one-time CC ring setup (~40us on
        # the first collective) while the input DMAs stream in
        ccw_sb = persist.tile([128, 512], BF, tag="ccw_sb")
        nc.vector.memset(ccw_sb, 0.0)
        ccw_in = dram.tile([128, 512], BF, name="ccw_in", tag="ccw_in")
        ccw_out = dram.tile([32, 512], BF, name="ccw_out", tag="ccw_out")
        nc.sync.dma_start(out=ccw_in[:], in_=ccw_sb)
        nc.gpsimd.collective_compute(
            "ReduceScatter", mybir.AluOpType.add,
            replica_groups=[[0, 1, 2, 3], [4, 5, 6, 7]],
            ins=[ccw_in.opt()], outs=[ccw_out.opt()])

        sbA = ctx.enter_context(tc.tile_pool(name="sbA", bufs=3))

        # ---------------- phase 1-2: load (pre-transposed on host), project
        ph12_ctx = contextlib.ExitStack()
        p12 = ph12_ctx.enter_context(tc.tile_pool(name="ph12", bufs=1))

        wq_sb = p12.tile([128, KD, C], PROJ_DT, tag="wq")
        wk_sb = p12.tile([128, KD, C], PROJ_DT, tag="wk")
        wv_sb = p12.tile([128, KD, C], PROJ_DT, tag="wv")
        xT_sb = p12.tile([128, KD, N], PROJ_DT, tag="xT")
        xT_src = xT_t.ap().rearrange("(k p) n -> p k n", p=128)
        nc.sync.dma_start(out=wq_sb, in_=wq_t.ap().rearrange("(k p) c -> p k c", p=128))
        nc.sync.dma_start(out=wk_sb, in_=wk_t.ap().rearrange("(k p) c -> p k c", p=128))
        nc.sync.dma_start(out=wv_sb, in_=wv_t.ap().rearrange("(k p) c -> p k c", p=128))
        for k in range(KD):
            nc.sync.dma_start(out=xT_sb[:, k, :], in_=xT_src[:, k, :])
        xT = [xT_sb[:, k, :] for k in range(KD)]

        posT_sb = p12.tile([128, 2, N], BF, tag="posT")
        nc.sync.dma_start(out=posT_sb,
                          in_=posT_t.ap().rearrange("(m p) n -> p m n", p=128))
        posT = [posT_sb[:, m, :] for m in range(2)]

        # projections: qT/kpT [128 c, N] (head pair hp at rows 64*(h%2))
        qT = [attnp.tile([128, N], ATT_DT, name=f"qT{m}", tag=f"qT{m}") for m in range(2)]
        kpT = [attnp.tile([128, N], ATT_DT, name=f"kpT{m}", tag=f"kpT{m}") for m in range(2)]
        V = [attnp.tile([128, HPC, DH + 1], ATT_DT, name=f"V{t}", tag=f"V{t}")
             for t in range(NT)]

        def proj_qkp(m):
            for s in range(NS):
                q_ps = psP.tile([128, 512], FP, tag="ps", name="q_ps")
                for k in range(KD):
                    nc.tensor.matmul(q_ps, wq_sb[:, k, m * 128:(m + 1) * 128],
                                     xT[k][:, s * 512:(s + 1) * 512],
                                     start=(k == 0), stop=(k == KD - 1))
                nc.vector.tensor_copy(out=qT[m][:, s * 512:(s + 1) * 512], in_=q_ps)
            for s in range(NS):
                kp_ps = psP.tile([128, 512], FP, tag="ps", name="kp_ps")
                for k in range(KD):
                    nc.tensor.matmul(kp_ps, wk_sb[:, k, m * 128:(m + 1) * 128],
                                     xT[k][:, s * 512:(s + 1) * 512],
                                     start=(k == 0), stop=(k == KD - 1))
                nc.vector.tensor_add(out=kpT[m][:, s * 512:(s + 1) * 512],
                                     in0=kp_ps, in1=posT[m][:, s * 512:(s + 1) * 512])

        proj_qkp(0)
        for t in range(NT):
            v_ps = psP.tile([128, C], FP, tag="ps", name="v_ps")
            for k in range(KD):
                nc.tensor.matmul(v_ps, xT[k][:, t * 128:(t + 1) * 128], wv_sb[:, k, :],
                                 start=(k == 0), stop=(k == KD - 1))
            nc.vector.tensor_copy(out=V[t][:, :, 0:DH],
                                  in_=v_ps.rearrange("p (h d) -> p h d", h=HPC))
            nc.vector.tensor_copy(out=V[t][:, :, DH:DH + 1],
                                  in_=onescol.broadcast_to([128, HPC, 1]))

        # ---------------- phases 3-4 interleaved per i-slice s -------------
        pools = {}

        wo_sb = persist.tile([128, 2, D], FPR, tag="wo")
        nc.sync.dma_start(out=wo_sb, in_=wo_t.ap().rearrange("(k p) d -> p k d", p=128))
        g_sb = persist.tile([128, D], FP, tag="g")
        b_sb = persist.tile([128, D], FP, tag="b")
        nc.gpsimd.dma_start(out=g_sb, in_=bcast_ap(g_t.ap(), 128))
        nc.gpsimd.dma_start(out=b_sb, in_=bcast_ap(bt_t.ap(), 128))
        eps_sb = persist.tile([128, 1], FP, tag="eps")
        nc.vector.memset(eps_sb, LN_EPS)

        OT = [attnp.tile([128, N], FPR, name=f"OT{m}", tag=f"OT{m}") for m in range(2)]
        OTU = [attnp.tile([128, N], FP, name=f"OTU{m}", tag=f"OTU{m}") for m in range(2)]
        oph = [dram.tile([R, D], BF, name=f"oph{s}", tag=f"oph{s}") for s in range(NS)]
        rsh = [dram.tile([128, D], BF, name=f"rsh{s}", tag=f"rsh{s}")
               for s in range(NS)]

        def attention(s, hp):
            ot_e = psO.tile([128, 512], FP, tag="ot", name="ot_e")
            ot_o = psO.tile([128, 512], FP, tag="ot", name="ot_o")
            for jt in range(NT):
                st = psC.tile([128, 1024], FP, tag="st", name="st")
                nc.tensor.matmul(st[:, 0:512],
                                 kpT[hp][0:64, jt * 128:(jt + 1) * 128],
                                 qT[hp][0:64, s * 512:(s + 1) * 512],
                                 start=True, stop=True)
                nc.tensor.matmul(st[:, 512:1024],
                                 kpT[hp][64:128, jt * 128:(jt + 1) * 128],
                                 qT[hp][64:128, s * 512:(s + 1) * 512],
                                 start=True, stop=True)
                ste = sbA.tile([128, 1024], ATT_DT, tag="ste", name="ste")
                nc.scalar.activation(out=ste, in_=st, func=AF.Exp, scale=SCALE)
                nc.tensor.matmul(ot_e[0:DH + 1, :], V[jt][:, 2 * hp, :],
                                 ste[:, 0:512],
                                 start=(jt == 0), stop=(jt == NT - 1))
                nc.tensor.matmul(ot_o[0:DH + 1, :], V[jt][:, 2 * hp + 1, :],
                                 ste[:, 512:1024],
                                 start=(jt == 0), stop=(jt == NT - 1))
            # evacuate PSUM immediately: unnormalized OT rows + colsum row to
            # SBUF (releases the ot accumulators within ~1.5us); the actual
            # softmax division happens later, overlapped with the next slice
            jobs = []
            for par, ot in ((0, ot_e), (1, ot_o)):
                csrow = sbA.tile([1, 512], FP, tag="csrow", name="csrow", bufs=8)
                nc.vector.tensor_copy(out=csrow, in_=ot[DH:DH + 1, :])
                dst = OT[hp][par * 64:par * 64 + DH, s * 512:(s + 1) * 512]
                dstu = OTU[hp][par * 64:par * 64 + DH, s * 512:(s + 1) * 512]
                nc.vector.tensor_copy(out=dstu, in_=ot[0:DH, :])
                jobs.append((dst, dstu, csrow, par))
            return jobs

        def normalize(jobs):
            for dst, dstu, csrow, par in jobs:
                csr = sbA.tile([1, 512], FP, tag="csr", name="csr", bufs=4)
                nc.vector.reciprocal_approx_fast(out=csr, in_=csrow)
                cs_d = dram.tile([1, 512], FP, tag="cs_d", name="cs_d", bufs=4)
                nc.sync.dma_start(out=cs_d[:], in_=csr)
                # rec must share its base partition with dst (DVE 2-SBUF rule)
                rec = sbA.tile([128, 512], FP, tag="rec", name="rec", bufs=4)
                recs = rec[par * 64:par * 64 + DH, :]
                cs_d_ap = cs_d.opt()
                # broadcast on the SP queue: gpsimd is the collective queue,
                # and routing this DMA there serializes each slice's softmax
                # normalization behind the previous slice's ReduceScatter
                nc.sync.dma_start(out=recs, in_=bass.AP(
                    tensor=cs_d_ap.tensor, offset=cs_d_ap.offset,
                    ap=[[0, DH]] + list(cs_d_ap.ap[1:])))
                nc.vector.tensor_mul(out=dst, in0=dstu, in1=recs)

        def outproj_rs_ln(s):
            sbB = pools["sbB"]
            # partial out-projection for this slice's 4 row blocks
            for it4 in range(4):
                it = s * 4 + it4
                op_sb = sbB.tile([128, D], BF, tag="op", name="op_sb")
                for nh in range(2):
                    op_ps = psP.tile([128, 512], FP, tag="ps", name="op_ps")
                    for kt in range(2):
                        nc.tensor.matmul(op_ps, OT[kt][:, it * 128:(it + 1) * 128],
                                         wo_sb[:, kt, nh * 512:(nh + 1) * 512],
                                         start=(kt == 0), stop=(kt == 1))
                    nc.vector.tensor_copy(out=op_sb[:, nh * 512:(nh + 1) * 512],
                                          in_=op_ps)
                nc.sync.dma_start(
                    out=oph[s][:].rearrange("(t p) d -> t p d", p=128)[it4],
                    in_=op_sb)
            nc.gpsimd.collective_compute(
                "ReduceScatter", mybir.AluOpType.add,
                replica_groups=[[0, 1, 2, 3], [4, 5, 6, 7]],
                ins=[oph[s].opt()], outs=[rsh[s].opt()])

        def ln_tail(s):
            # residual + LayerNorm on this core's 128-row chunk; deferred to
            # the tail so no engine queue ever blocks on a ReduceScatter
            # mid-pipeline (in-order queues propagate such a wait everywhere)
            sbB = pools["sbB"]
            xr = sbB.tile([128, D], FP, tag="xr", name="xr")
            rd = sbB.tile([128, D], FP, tag="rd", name="rd")
            rs_sb = sbB.tile([128, D], BF, tag="rsld", name="rs_sb")
            nc.sync.dma_start(out=rd, in_=res_tiles[s])
            nc.sync.dma_start(out=rs_sb, in_=rsh[s][:])
            nc.vector.tensor_add(out=xr, in0=rs_sb, in1=rd)
            stats = sbB.tile([128, 2, 6], FP, tag="stats", name="stats")
            mv = sbB.tile([128, 2], FP, tag="mv", name="mv")
            nc.vector.bn_stats(out=stats[:, 0, :], in_=xr[:, 0:512])
            nc.vector.bn_stats(out=stats[:, 1, :], in_=xr[:, 512:1024])
            nc.vector.bn_aggr(out=mv, in_=stats)
            # rstd = exp(-0.5*ln(var+eps)); Log/Exp share one ACT table set
            nc.scalar.activation(out=mv[:, 1:2], in_=mv[:, 1:2], func=AF.Ln,
                                 bias=eps_sb, scale=1.0)
            nc.scalar.activation(out=mv[:, 1:2], in_=mv[:, 1:2], func=AF.Exp,
                                 scale=-0.5)
            nc.vector.tensor_scalar(out=xr, in0=xr,
                                    scalar1=mv[:, 0:1], scalar2=mv[:, 1:2],
                                    op0=mybir.AluOpType.subtract,
                                    op1=mybir.AluOpType.mult)
            nc.vector.tensor_mul(out=xr, in0=xr, in1=g_sb)
            nc.vector.tensor_add(out=xr, in0=xr, in1=b_sb)
            nc.sync.dma_start(out=out_tiles[s], in_=xr)

        for s in range(NS):
            jobs = attention(s, 0)
            if s == 0:
                proj_qkp(1)  # overlaps first attention slice on other engines
            jobs += attention(s, 1)
            normalize(jobs)
            if s == 0:
                # x/pos/weight staging no longer needed; free its SBUF before
                # opening the out-proj/LN pool
                ph12_ctx.close()
                pools["sbB"] = ctx.enter_context(tc.tile_pool(name="sbB", bufs=2))
            outproj_rs_ln(s)
        for s in range(NS):
            # logical-priority override: keep every RS-dependent instruction
            # at the end of each engine stream, so no in-order engine queue
            # ever blocks mid-pipeline waiting for a ReduceScatter
            with tc.tile_wait_until(1.0 + 0.01 * s):
                ln_tail(s)

    nc.compile()
    return nc


_NC = None
_last_in_maps = None


def kernel(**inputs) -> np.ndarray:
    global _NC, _last_in_maps
    if _NC is None:
        _NC = build()
    nc = _NC

    q_s = np.asarray(inputs["q_s"], np.float32)
    pos = np.asarray(inputs["pos_emb"], np.float32)
    Wq = np.asarray(inputs["Wq"], np.float32)
    Wk = np.asarray(inputs["Wk"], np.float32)
    Wv = np.asarray(inputs["Wv"], np.float32)
    Wo = np.asarray(inputs["Wo"], np.float32)
    bo = np.asarray(inputs["bo"], np.float32)
    ln_g = np.asarray(inputs["ln_g"], np.float32)
    ln_b = np.asarray(inputs["ln_b"], np.float32)

    in_maps = []
    for c in range(NCORES):
        b, g = divmod(c, GRP)
        cs = slice(g * C, (g + 1) * C)
        resid = np.concatenate(
            [q_s[b][512 * s + 128 * g: 512 * s + 128 * (g + 1)] for s in range(NS)],
            axis=0) + bo[None, :]
        bf = ml_dtypes.bfloat16
        in_maps.append({
            "xT": np.ascontiguousarray(q_s[b].T.astype(bf)),
            "posT": np.ascontiguousarray(pos[b][:, cs].T.astype(bf)),
            "wq": np.ascontiguousarray(Wq[:, cs].astype(bf)),
            "wk": np.ascontiguousarray(Wk[:, cs].astype(bf)),
            "wv": np.ascontiguousarray(Wv[:, cs].astype(bf)),
            "wo": np.ascontiguousarray(Wo[cs, :]),
            "resid": np.ascontiguousarray(resid),
            "ln_g": ln_g,
            "ln_b": ln_b,
        })

    _last_in_maps = in_maps
    res = run_bass_kernel_spmd(nc, in_maps, list(range(NCORES)))
    out = np.empty((B, N, D), np.float32)
    for c in range(NCORES):
        b, g = divmod(c, GRP)
        o = res.results[c]["out"]
        for s in range(NS):
            out[b, 512 * s + 128 * g: 512 * s + 128 * (g + 1), :] = \
                o[128 * s:128 * (s + 1)]
    return out



# revision 23
# speedup vs baseline: 1.3126x; 1.0882x over previous
"""BoT multi-head attention block (QKV proj + content/position attention +
out-proj + residual + LayerNorm) on 8 Trainium2 NeuronCores.

Sharding: tensor-parallel over heads (4 heads/core) x batch (2 batches, 4
cores each).  Each core computes q/k/v projections for its 256 feature
columns, full attention for its 4 heads, and a partial out-projection;
partials are summed with row-chunked ReduceScatters over each 4-core batch
group (overlapped with attention of later chunks), after which each core
applies residual + LayerNorm to its 4x128 rows.

Layout trick: attention logits are computed TRANSPOSED (j on partitions, i
free) so the softmax numerator matmul (P^T moving, V stationary) needs no
transpose of the probability matrix; an extra all-ones column in the
stationary V supplies the softmax denominator for free.  Host passes x and
pos pre-transposed.  Projections/out-proj run in fp32r, attention matmuls
in bf16 (1 cycle/col vs 2 for fp32r).
"""

import contextlib
import os
import sys

os.environ.setdefault("MYCRO_LOCAL_CACHE", "1")
for _p in ("/opt/trn_rl_repo",):
    if os.path.isdir(_p) and _p not in sys.path:
        sys.path.append(_p)

import ml_dtypes
import numpy as np

import concourse.bass as bass
from concourse import bacc
import concourse.mybir as mybir
import concourse.tile as tile
from concourse.bass_utils import run_bass_kernel_spmd

FP = mybir.dt.float32
FPR = mybir.dt.float32r
BF = mybir.dt.bfloat16
AF = mybir.ActivationFunctionType

B, N, D, H = 2, 2048, 1024, 16
NCORES = 8
GRP = 4                # cores per batch group
HPC = H // GRP         # heads per core = 4
C = D // GRP           # feature cols per core = 256
R = N // GRP           # output rows per core = 512
DH = D // H            # head dim = 64
SCALE = DH ** -0.5
LN_EPS = 1e-5

NT = N // 128          # 16 row tiles
KD = D // 128          # 8 contraction tiles over D
NS = N // 512          # 4 i-slices

ATT_DT = BF            # dtype of attention matmul operands
PROJ_DT = BF           # dtype of projection inputs (xT, wq/wk/wv)


def build():
    nc = bacc.Bacc("TRN2", target_bir_lowering=False, num_devices=NCORES)

    # fp32r-typed inputs are plain fp32 bits; typing them fp32r lets HWDGE
    # load them with no cast while satisfying the fp32r-producer rule.
    xT_t = nc.dram_tensor("xT", [D, N], PROJ_DT, kind="ExternalInput")
    posT_t = nc.dram_tensor("posT", [C, N], BF, kind="ExternalInput")
    wq_t = nc.dram_tensor("wq", [D, C], PROJ_DT, kind="ExternalInput")
    wk_t = nc.dram_tensor("wk", [D, C], PROJ_DT, kind="ExternalInput")
    wv_t = nc.dram_tensor("wv", [D, C], PROJ_DT, kind="ExternalInput")
    wo_t = nc.dram_tensor("wo", [C, D], FPR, kind="ExternalInput")
    res_t = nc.dram_tensor("resid", [R, D], BF, kind="ExternalInput")
    g_t = nc.dram_tensor("ln_g", [D], FP, kind="ExternalInput")
    bt_t = nc.dram_tensor("ln_b", [D], FP, kind="ExternalInput")
    out_t = nc.dram_tensor("out", [R, D], FP, kind="ExternalOutput")

    res_tiles = res_t.ap().rearrange("(t p) d -> t p d", p=128)
    out_tiles = out_t.ap().rearrange("(t p) d -> t p d", p=128)

    def bcast_ap(ap, parts):
        return bass.AP(tensor=ap.tensor, offset=ap.offset,
                       ap=[[0, parts]] + list(ap.ap))

    with tile.TileContext(nc) as tc, contextlib.ExitStack() as ctx:
        persist = ctx.enter_context(tc.tile_pool(name="persist", bufs=1))
        attnp = ctx.enter_context(tc.tile_pool(name="attnp", bufs=1))
        psP = ctx.enter_context(tc.tile_pool(name="psP", bufs=2, space="PSUM"))
        psO = ctx.enter_context(tc.tile_pool(name="psO", bufs=2, space="PSUM"))
        psC = ctx.enter_context(tc.tile_pool(name="psC", bufs=2, space="PSUM"))
        dram = ctx.enter_context(tc.tile_pool(name="dram", bufs=1, space="DRAM"))

        ones64 = persist.tile([1, DH], FP, tag="ones64")
        nc.vector.memset(ones64, 1.0)
        ones64b = persist.tile([1, DH], BF, tag="ones64b")
        nc.vector.memset(ones64b, 1.0)
        onescol = persist.tile([128, 1], FP, tag="onescol")
        nc.vector.memset(onescol, 1.0)

        # tiny warmup collective: absorbs one-time CC ring setup (~40us on
        # the first collective) while the input DMAs stream in
        ccw_sb = persist.tile([128, 512], BF, tag="ccw_sb")
        nc.vector.memset(ccw_sb, 0.0)
        ccw_in = dram.tile([128, 512], BF, name="ccw_in", tag="ccw_in")
        ccw_out = dram.tile([32, 512], BF, name="ccw_out", tag="ccw_out")
        nc.sync.dma_start(out=ccw_in[:], in_=ccw_sb)
        nc.gpsimd.collective_compute(
            "ReduceScatter", mybir.AluOpType.add,
            replica_groups=[[0, 1, 2, 3], [4, 5, 6, 7]],
            ins=[ccw_in.opt()], outs=[ccw_out.opt()])

        sbA = ctx.enter_context(tc.tile_pool(name="sbA", bufs=3))

        # ---------------- phase 1-2: load (pre-transposed on host), project
        ph12_ctx = contextlib.ExitStack()
        p12 = ph12_ctx.enter_context(tc.tile_pool(name="ph12", bufs=1))

        wq_sb = p12.tile([128, KD, C], PROJ_DT, tag="wq")
        wk_sb = p12.tile([128, KD, C], PROJ_DT, tag="wk")
        wv_sb = p12.tile([128, KD, C], PROJ_DT, tag="wv")
        xT_sb = p12.tile([128, KD, N], PROJ_DT, tag="xT")
        xT_src = xT_t.ap().rearrange("(k p) n -> p k n", p=128)
        nc.sync.dma_start(out=wq_sb, in_=wq_t.ap().rearrange("(k p) c -> p k c", p=128))
        nc.sync.dma_start(out=wk_sb, in_=wk_t.ap().rearrange("(k p) c -> p k c", p=128))
        nc.sync.dma_start(out=wv_sb, in_=wv_t.ap().rearrange("(k p) c -> p k c", p=128))
        for k in range(KD):
            nc.sync.dma_start(out=xT_sb[:, k, :], in_=xT_src[:, k, :])
        xT = [xT_sb[:, k, :] for k in range(KD)]

        posT_sb = p12.tile([128, 2, N], BF, tag="posT")
        nc.sync.dma_start(out=posT_sb,
                          in_=posT_t.ap().rearrange("(m p) n -> p m n", p=128))
        posT = [posT_sb[:, m, :] for m in range(2)]

        # projections: qT/kpT [128 c, N] (head pair hp at rows 64*(h%2))
        qT = [attnp.tile([128, N], ATT_DT, name=f"qT{m}", tag=f"qT{m}") for m in range(2)]
        kpT = [attnp.tile([128, N], ATT_DT, name=f"kpT{m}", tag=f"kpT{m}") for m in range(2)]
        V = [attnp.tile([128, HPC, DH + 1], ATT_DT, name=f"V{t}", tag=f"V{t}")
             for t in range(NT)]

        def proj_qkp(m):
            for s in range(NS):
                q_ps = psP.tile([128, 512], FP, tag="ps", name="q_ps")
                for k in range(KD):
                    nc.tensor.matmul(q_ps, wq_sb[:, k, m * 128:(m + 1) * 128],
                                     xT[k][:, s * 512:(s + 1) * 512],
                                     start=(k == 0), stop=(k == KD - 1))
                nc.vector.tensor_copy(out=qT[m][:, s * 512:(s + 1) * 512], in_=q_ps)
            for s in range(NS):
                kp_ps = psP.tile([128, 512], FP, tag="ps", name="kp_ps")
                for k in range(KD):
                    nc.tensor.matmul(kp_ps, wk_sb[:, k, m * 128:(m + 1) * 128],
                                     xT[k][:, s * 512:(s + 1) * 512],
                                     start=(k == 0), stop=(k == KD - 1))
                nc.vector.tensor_add(out=kpT[m][:, s * 512:(s + 1) * 512],
                                     in0=kp_ps, in1=posT[m][:, s * 512:(s + 1) * 512])

        proj_qkp(0)
        for t in range(NT):
            v_ps = psP.tile([128, C], FP, tag="ps", name="v_ps")
            for k in range(KD):
                nc.tensor.matmul(v_ps, xT[k][:, t * 128:(t + 1) * 128], wv_sb[:, k, :],
                                 start=(k == 0), stop=(k == KD - 1))
            nc.vector.tensor_copy(out=V[t][:, :, 0:DH],
                                  in_=v_ps.rearrange("p (h d) -> p h d", h=HPC))
            nc.vector.tensor_copy(out=V[t][:, :, DH:DH + 1],
                                  in_=onescol.broadcast_to([128, HPC, 1]))

        # ---------------- phases 3-4 interleaved per i-slice s -------------
        pools = {}

        wo_sb = persist.tile([128, 2, D], FPR, tag="wo")
        nc.sync.dma_start(out=wo_sb, in_=wo_t.ap().rearrange("(k p) d -> p k d", p=128))
        g_sb = persist.tile([128, D], FP, tag="g")
        b_sb = persist.tile([128, D], FP, tag="b")
        nc.gpsimd.dma_start(out=g_sb, in_=bcast_ap(g_t.ap(), 128))
        nc.gpsimd.dma_start(out=b_sb, in_=bcast_ap(bt_t.ap(), 128))
        eps_sb = persist.tile([128, 1], FP, tag="eps")
        nc.vector.memset(eps_sb, LN_EPS)

        OT = [attnp.tile([128, N], FPR, name=f"OT{m}", tag=f"OT{m}") for m in range(2)]
        OTU = [attnp.tile([128, N], FP, name=f"OTU{m}", tag=f"OTU{m}") for m in range(2)]
        oph = [dram.tile([R, D], BF, name=f"oph{s}", tag=f"oph{s}") for s in range(NS)]
        ophh = [dram.tile([R // 2, D], BF, name=f"ophh{h}", tag=f"ophh{h}")
                for h in range(2)]
        rsh = [dram.tile([128, D], BF, name=f"rsh{s}", tag=f"rsh{s}")
               for s in range(NS)]

        def attention(s, hp):
            ot_e = psO.tile([128, 512], FP, tag="ot", name="ot_e")
            ot_o = psO.tile([128, 512], FP, tag="ot", name="ot_o")
            for jt in range(NT):
                st = psC.tile([128, 1024], FP, tag="st", name="st")
                nc.tensor.matmul(st[:, 0:512],
                                 kpT[hp][0:64, jt * 128:(jt + 1) * 128],
                                 qT[hp][0:64, s * 512:(s + 1) * 512],
                                 start=True, stop=True)
                nc.tensor.matmul(st[:, 512:1024],
                                 kpT[hp][64:128, jt * 128:(jt + 1) * 128],
                                 qT[hp][64:128, s * 512:(s + 1) * 512],
                                 start=True, stop=True)
                ste = sbA.tile([128, 1024], ATT_DT, tag="ste", name="ste")
                nc.scalar.activation(out=ste, in_=st, func=AF.Exp, scale=SCALE)
                nc.tensor.matmul(ot_e[0:DH + 1, :], V[jt][:, 2 * hp, :],
                                 ste[:, 0:512],
                                 start=(jt == 0), stop=(jt == NT - 1))
                nc.tensor.matmul(ot_o[0:DH + 1, :], V[jt][:, 2 * hp + 1, :],
                                 ste[:, 512:1024],
                                 start=(jt == 0), stop=(jt == NT - 1))
            # evacuate PSUM immediately: unnormalized OT rows + colsum row to
            # SBUF (releases the ot accumulators within ~1.5us); the actual
            # softmax division happens later, overlapped with the next slice
            jobs = []
            for par, ot in ((0, ot_e), (1, ot_o)):
                csrow = sbA.tile([1, 512], FP, tag="csrow", name="csrow", bufs=8)
                nc.vector.tensor_copy(out=csrow, in_=ot[DH:DH + 1, :])
                dst = OT[hp][par * 64:par * 64 + DH, s * 512:(s + 1) * 512]
                dstu = OTU[hp][par * 64:par * 64 + DH, s * 512:(s + 1) * 512]
                nc.vector.tensor_copy(out=dstu, in_=ot[0:DH, :])
                jobs.append((dst, dstu, csrow, par))
            return jobs

        def normalize(jobs):
            for dst, dstu, csrow, par in jobs:
                # reciprocal of the column sums, then broadcast down 64
                # partitions with a rank-1 matmul (ones[64] x csr) into PSUM:
                # no DRAM round-trip, no DMA queue involvement at all
                csr = sbA.tile([1, 512], FP, tag="csr", name="csr", bufs=4)
                nc.vector.reciprocal_approx_fast(out=csr, in_=csrow)
                csrb = sbA.tile([1, 512], BF, tag="csrb", name="csrb", bufs=4)
                nc.vector.tensor_copy(out=csrb, in_=csr)
                rec_ps = psP.tile([128, 512], FP, tag="ps", name="rec_ps")
                nc.tensor.matmul(rec_ps[par * 64:par * 64 + DH, :],
                                 ones64b, csrb, start=True, stop=True)
                nc.vector.tensor_mul(out=dst, in0=dstu,
                                     in1=rec_ps[par * 64:par * 64 + DH, :])

        def outproj_rs_ln(s):
            sbB = pools["sbB"]
            # partial out-projection for this slice's 4 row blocks
            for it4 in range(4):
                it = s * 4 + it4
                op_sb = sbB.tile([128, D], BF, tag="op", name="op_sb")
                for nh in range(2):
                    op_ps = psP.tile([128, 512], FP, tag="ps", name="op_ps")
                    for kt in range(2):
                        nc.tensor.matmul(op_ps, OT[kt][:, it * 128:(it + 1) * 128],
                                         wo_sb[:, kt, nh * 512:(nh + 1) * 512],
                                         start=(kt == 0), stop=(kt == 1))
                    nc.vector.tensor_copy(out=op_sb[:, nh * 512:(nh + 1) * 512],
                                          in_=op_ps)
                if s < NS - 1:
                    nc.sync.dma_start(
                        out=oph[s][:].rearrange("(t p) d -> t p d", p=128)[it4],
                        in_=op_sb)
                else:
                    # final slice: store block halves into separate tiles so
                    # the tail RS can be split into two contiguous collectives
                    for h in range(2):
                        nc.sync.dma_start(
                            out=ophh[h][:].rearrange("(t p) d -> t p d",
                                                     p=64)[it4],
                            in_=op_sb[h * 64:(h + 1) * 64, :])
            if s < NS - 1:
                nc.gpsimd.collective_compute(
                    "ReduceScatter", mybir.AluOpType.add,
                    replica_groups=[[0, 1, 2, 3], [4, 5, 6, 7]],
                    ins=[oph[s].opt()], outs=[rsh[s].opt()])
            else:
                # the final RS is fully exposed in the tail: split it in two
                # so LN work can begin while the second half transfers
                out_halves = rsh[s][:].rearrange("(h p) d -> h p d", h=2)
                for h in range(2):
                    nc.gpsimd.collective_compute(
                        "ReduceScatter", mybir.AluOpType.add,
                        replica_groups=[[0, 1, 2, 3], [4, 5, 6, 7]],
                        ins=[ophh[h].opt()], outs=[out_halves[h]])

        def ln_tail(s):
            # residual + LayerNorm on this core's 128-row chunk; deferred to
            # the tail so no engine queue ever blocks on a ReduceScatter
            # mid-pipeline (in-order queues propagate such a wait everywhere)
            sbB = pools["sbB"]
            xr = sbB.tile([128, D], FP, tag="xr", name="xr")
            rd = pools["rd"][s]
            rs_sb = sbB.tile([128, D], BF, tag="rsld", name="rs_sb")
            nc.sync.dma_start(out=rs_sb, in_=rsh[s][:])
            nc.vector.tensor_add(out=xr, in0=rs_sb, in1=rd)
            stats = sbB.tile([128, 2, 6], FP, tag="stats", name="stats")
            mv = sbB.tile([128, 2], FP, tag="mv", name="mv")
            nc.vector.bn_stats(out=stats[:, 0, :], in_=xr[:, 0:512])
            nc.vector.bn_stats(out=stats[:, 1, :], in_=xr[:, 512:1024])
            nc.vector.bn_aggr(out=mv, in_=stats)
            # rstd = exp(-0.5*ln(var+eps)); Log/Exp share one ACT table set
            nc.scalar.activation(out=mv[:, 1:2], in_=mv[:, 1:2], func=AF.Ln,
                                 bias=eps_sb, scale=1.0)
            nc.scalar.activation(out=mv[:, 1:2], in_=mv[:, 1:2], func=AF.Exp,
                                 scale=-0.5)
            nc.vector.tensor_scalar(out=xr, in0=xr,
                                    scalar1=mv[:, 0:1], scalar2=mv[:, 1:2],
                                    op0=mybir.AluOpType.subtract,
                                    op1=mybir.AluOpType.mult)
            nc.vector.tensor_mul(out=xr, in0=xr, in1=g_sb)
            nc.vector.tensor_add(out=xr, in0=xr, in1=b_sb)
            nc.sync.dma_start(out=out_tiles[s], in_=xr)

        for s in range(NS):
            jobs = attention(s, 0)
            if s == 0:
                proj_qkp(1)  # overlaps first attention slice on other engines
            jobs += attention(s, 1)
            normalize(jobs)
            if s == 0:
                # x/pos/weight staging no longer needed; free its SBUF before
                # opening the out-proj/LN pool
                ph12_ctx.close()
                pools["sbB"] = ctx.enter_context(tc.tile_pool(name="sbB", bufs=2))
                # prefetch all residual tiles now; they depend on nothing
                pools["rd"] = [pools["sbB"].tile([128, D], BF, tag=f"rd{t}",
                                                 name=f"rd{t}") for t in range(NS)]
                for t in range(NS):
                    nc.sync.dma_start(out=pools["rd"][t], in_=res_tiles[t])
            outproj_rs_ln(s)
        for s in range(NS):
            # logical-priority override: keep every RS-dependent instruction
            # at the end of each engine stream, so no in-order engine queue
            # ever blocks mid-pipeline waiting for a ReduceScatter
            with tc.tile_wait_until(1.0 + 0.01 * s):
                ln_tail(s)

    nc.compile()
    return nc


_NC = None
_last_in_maps = None


def kernel(**inputs) -> np.ndarray:
    global _NC, _last_in_maps
    if _NC is None:
        _NC = build()
    nc = _NC

    q_s = np.asarray(inputs["q_s"], np.float32)
    pos = np.asarray(inputs["pos_emb"], np.float32)
    Wq = np.asarray(inputs["Wq"], np.float32)
    Wk = np.asarray(inputs["Wk"], np.float32)
    Wv = np.asarray(inputs["Wv"], np.float32)
    Wo = np.asarray(inputs["Wo"], np.float32)
    bo = np.asarray(inputs["bo"], np.float32)
    ln_g = np.asarray(inputs["ln_g"], np.float32)
    ln_b = np.asarray(inputs["ln_b"], np.float32)

    in_maps = []
    for c in range(NCORES):
        b, g = divmod(c, GRP)
        cs = slice(g * C, (g + 1) * C)
        resid = np.concatenate(
            [q_s[b][512 * s + 128 * g: 512 * s + 128 * (g + 1)] for s in range(NS)],
            axis=0) + bo[None, :]
        bf = ml_dtypes.bfloat16
        in_maps.append({
            "xT": np.ascontiguousarray(q_s[b].T.astype(bf)),
            "posT": np.ascontiguousarray(pos[b][:, cs].T.astype(bf)),
            "wq": np.ascontiguousarray(Wq[:, cs].astype(bf)),
            "wk": np.ascontiguousarray(Wk[:, cs].astype(bf)),
            "wv": np.ascontiguousarray(Wv[:, cs].astype(bf)),
            "wo": np.ascontiguousarray(Wo[cs, :]),
            "resid": np.ascontiguousarray(resid.astype(bf)),
            "ln_g": ln_g,
            "ln_b": ln_b,
        })

    _last_in_maps = in_maps
    res = run_bass_kernel_spmd(nc, in_maps, list(range(NCORES)))
    out = np.empty((B, N, D), np.float32)
    for c in range(NCORES):
        b, g = divmod(c, GRP)
        o = res.results[c]["out"]
        for s in range(NS):
            out[b, 512 * s + 128 * g: 512 * s + 128 * (g + 1), :] = \
                o[128 * s:128 * (s + 1)]
    return out

